# revision 28
# baseline (speedup 1.0000x reference)
"""Trainium2 Bass kernel v2 for nn_Attention (B=1, C=64, 12^3 spatial, 32 heads, d=2).

Sharding: 32 heads over 8 cores (4 heads/core), tensor-parallel w_proj with
host-side partial sum (no collectives).

Key structure per core (vs v1 baseline):
- exp work split between ScalarE (true exp activation) and VectorE
  (Schraudolph exp2 bit-trick: int16(s*A+B) bitcast to bf16, ~3% rel err on
  a tunable fraction of key chunks) so the two engines share the 12M-element
  softmax exponential instead of serializing on ScalarE.
- S tiles pack a head PAIR per PSUM tile [128, 2x512] so one exp instruction
  covers 1024 elements/partition (fewer, larger activations).
- U/Z accumulated flash-style via V'=[V0,V1,1] matmuls; PSUM pre-zeroed by a
  zero-weight matmul (PE) so downstream full-tile reads are initialized.
- 1/Z via ACT ln then exp(-x) (same activation table set as exp -> no
  table switch), duplicated to the two U rows per head by broadcast DMAs.
- proj computed as y^T = Wp'^T @ O_sp (stationary Wp', moving normalized O),
  bias added via broadcast tensor_tensor; output is y^T [64, N] summed on host.
- a dummy exp right after startup forces the ACT table load off the
  critical path.

reps>1 wraps the body in a hardware For_i loop (HW timing measurements).

Self-contained: hardcodes all shapes.
"""

import numpy as np
import ml_dtypes

import concourse.bass as bass
import concourse.bacc as bacc
import concourse.mybir as mybir
from concourse import tile
from concourse.bass_utils import run_bass_kernel_spmd

C = 64
N = 1728  # 12*12*12
NCORES = 8
HLOC = 4
SCALE = float(2.0 ** -0.5)
LOG2E = float(np.log2(np.e))
# Schraudolph bf16-exp2 constants: bits_i16 = s*A + B, bits viewed as bf16
A_SCH = SCALE * LOG2E * 128.0
B_SCH = (127.0 - 0.04367) * 128.0 + 0.5

KCS = [(i * 128, 128) for i in range(13)] + [(1664, 64)]
NKC = len(KCS)
QTS = [(0, 512), (512, 512), (1024, 512), (1536, 192)]
SEGS = [(0, 512), (512, 512), (1024, 512), (1536, 192)]  # qkv segments

F32 = mybir.dt.float32
BF16 = mybir.dt.bfloat16
I16 = mybir.dt.int16

F_DVE = 0.45      # fraction of exp tiles on VectorE (Schraudolph)
QK_ACT = 0.5      # fraction of qk PSUM->SBUF copies on ScalarE


def _mk_assign(n, frac):
    """Error-diffused boolean assignment list: True -> alt engine."""
    out, acc = [], 0.0
    for _ in range(n):
        acc += frac
        if acc >= 1.0 - 1e-9:
            acc -= 1.0
            out.append(True)
        else:
            out.append(False)
    return out


def build_nc(f_dve=F_DVE, qk_act=QK_ACT, reps=1):
    nc = bacc.Bacc(None)

    x2 = nc.declare_dram_parameter("x2", [C, N], BF16, isOutput=False)
    wqkv = nc.declare_dram_parameter("wqkv", [C, 24], BF16, isOutput=False)
    wpa = nc.declare_dram_parameter("wpa", [128, C], BF16, isOutput=False)
    bcol = nc.declare_dram_parameter("bcol", [C, 1], F32, isOutput=False)
    yt = nc.declare_dram_parameter("yt", [C, N], F32, isOutput=True)

    with tile.TileContext(nc) as tc:
        with (
            tc.tile_pool(name="const", bufs=1) as cpool,
            tc.tile_pool(name="epool", bufs=5) as epool,
            tc.tile_pool(name="ebpool", bufs=5) as ebpool,
            tc.tile_pool(name="tails", bufs=2) as tailpool,
            tc.tile_pool(name="ps_s", bufs=3, space=bass.MemorySpace.PSUM) as ps_s,
            tc.tile_pool(name="ps_u", bufs=2, space=bass.MemorySpace.PSUM) as ps_u,
        ):
            ps_m = ps_u  # pu/psv/qkv/ytp share the two 1-bank slots
            x_sb = cpool.tile([C, N], BF16, name="x_sb")
            w_sb = cpool.tile([C, 24], BF16, name="w_sb")
            wp_sb = cpool.tile([128, C], BF16, name="wp_sb")
            b_sb = cpool.tile([C, 1], F32, name="b_sb")
            qT = cpool.tile([128, N], BF16, name="qT")
            kT = cpool.tile([128, N], BF16, name="kT")
            q8 = cpool.tile([8, N], BF16, name="q8")
            k8 = cpool.tile([8, N], BF16, name="k8")
            vp = cpool.tile([128, NKC * HLOC * 3], BF16, name="vp")
            zrow = cpool.tile([1, 128], BF16, name="zrow")
            zdum = cpool.tile([1, 512], BF16, name="zdum")
            scr = cpool.tile([1, 8], F32, name="scr")
            zz = cpool.tile([128, N], F32, name="zz")
            osp = cpool.tile([128, N], BF16, name="osp")
            yt_sb = cpool.tile([C, N], F32, name="yt_sb")
            vp_v = vp[:].rearrange("p (kc h d) -> p kc h d", h=HLOC, d=3)

            def init():
                # one-time constants, hoisted out of the rep loop
                nc.gpsimd.memset(zrow[:], 0.0)
                nc.gpsimd.memset(zdum[:], 0.0)
                # dummy exp: forces the ACT table load at t~0
                nc.scalar.activation(
                    scr[0:1, 0:2], zrow[0:1, 0:2],
                    mybir.ActivationFunctionType.Exp,
                )
                nc.sync.dma_start(out=w_sb[:], in_=wqkv[:])
                nc.sync.dma_start(out=wp_sb[:], in_=wpa[:])
                nc.sync.dma_start(out=b_sb[:], in_=bcol[:])
                # zz rows outside the dup-DMA bands stay zero forever; the
                # vp ones-column is never touched by the per-iter copies
                nc.gpsimd.memset(zz[:], 0.0)
                nc.gpsimd.memset(vp_v[:, :, :, 2:3], 1.0)

            def body():
                exp_assign = _mk_assign(len(QTS) * NKC * 2, f_dve)
                qk_assign = _mk_assign(2 * len(SEGS), qk_act)
                exp_i = [0]
                qk_i = [0]

                nc.sync.dma_start(out=x_sb[:, 0:512], in_=x2[:, 0:512])
                nc.sync.dma_start(out=x_sb[:, 512:N], in_=x2[:, 512:N])

                # ---- V': per key chunk, v rows [kn, 8] -> vp packed bf16 ----
                psv = ps_m.tile([128, 512], F32, tag="pu", name="psv")
                for kc, (ko, kn) in enumerate(KCS):
                    nc.tensor.matmul(
                        psv[:kn, 8 * kc : 8 * kc + 8],
                        x_sb[:, ko : ko + kn],
                        w_sb[:, 16:24],
                        start=True, stop=True,
                    )
                vsrc = psv[:, 0:104].rearrange(
                    "p (kc h d) -> p kc h d", h=HLOC, d=2
                )
                nc.vector.tensor_copy(vp_v[:, 0:13, :, 0:2], vsrc)
                vtail = psv[:64, 104:112].rearrange("p (h d) -> p h d", d=2)
                nc.vector.tensor_copy(vp_v[:64, 13, :, 0:2], vtail)

                # ---- qkv: one [8, segn] matmul per seg; stage to an [8, N]
                # sbuf tile, then DMA-distribute head rows 2h -> band 32h ----
                def qkv_seg(which, dst8, seg):
                    so, sn = SEGS[seg]
                    wofs = 0 if which == "q" else 8
                    ps = ps_m.tile([128, 512], F32, tag="pu", name="ps_qk")
                    nc.tensor.matmul(
                        ps[0:8, :sn],
                        w_sb[:, wofs : wofs + 8],
                        x_sb[:, so : so + sn],
                        start=True, stop=True,
                    )
                    on_act = qk_assign[qk_i[0]]
                    qk_i[0] += 1
                    src = ps[0:8, :sn]
                    dd = dst8[0:8, so : so + sn]
                    if on_act:
                        nc.scalar.copy(dd, src)
                    else:
                        nc.vector.tensor_copy(dd, src)

                def band_dist(dst, src8, so, sn):
                    """DMA rows 2h:2h+2 of src8 -> rows 32h:32h+2 of dst."""
                    for h in range(HLOC):
                        dma = nc.sync.dma_start if h % 2 == 0 else nc.gpsimd.dma_start
                        dma(
                            out=dst[32 * h : 32 * h + 2, so : so + sn],
                            in_=src8[2 * h : 2 * h + 2, so : so + sn],
                        )

                qkv_seg("k", k8, 0)
                qkv_seg("q", q8, 0)
                band_dist(kT, k8, 0, 512)
                band_dist(qT, q8, 0, 512)

                # U matmuls run one key-chunk behind the exps — and the
                # pipeline carries ACROSS qtile boundaries — so the in-order
                # PE queue never fences on the current chunk's exp.
                pend = [None]  # (pu, qn, kc, kn, es, last)

                def emit_u(pu, qn, kc, kn, es, last):
                    for h in range(HLOC):
                        nc.tensor.matmul(
                            pu[32 * h : 32 * h + 3, :qn],
                            vp_v[:kn, kc, h, :],
                            es[h // 2][
                                :kn, 512 * (h % 2) : 512 * (h % 2) + qn
                            ],
                            start=False,
                            stop=(last and h == HLOC - 1),
                            skip_group_check=True,
                            tile_position=(0, 32 * h),
                        )

                def qtile(qt, boundary_work):
                    qo, qn = QTS[qt]
                    pu = ps_u.tile([128, 512], F32, tag="pu", name="pu")
                    # zero the U bank via a zero-weight matmul (starts group)
                    nc.tensor.matmul(
                        pu[:, :qn], zrow[:], zdum[0:1, :qn],
                        start=True, stop=False, skip_group_check=True,
                    )
                    for kc, (ko, kn) in enumerate(KCS):
                        es = []
                        for pr in range(2):
                            ha, hb = 2 * pr, 2 * pr + 1
                            st = ps_s.tile([128, 1024], F32, tag="s", name="st")
                            nc.tensor.matmul(
                                st[:kn, 0:qn],
                                kT[32 * ha : 32 * ha + 2, ko : ko + kn],
                                qT[32 * ha : 32 * ha + 2, qo : qo + qn],
                                start=True, stop=True,
                                tile_position=(32 * ha, 0),
                            )
                            nc.tensor.matmul(
                                st[:kn, 512 : 512 + qn],
                                kT[32 * hb : 32 * hb + 2, ko : ko + kn],
                                qT[32 * hb : 32 * hb + 2, qo : qo + qn],
                                start=True, stop=True,
                                tile_position=(32 * hb, 0),
                            )
                            on_dve = exp_assign[exp_i[0]]
                            exp_i[0] += 1
                            sv = st[:kn].rearrange("k (p q) -> k p q", p=2)[
                                :, :, :qn
                            ]
                            if on_dve:
                                eb = ebpool.tile(
                                    [128, 1024], I16, tag="eb", name="eb"
                                )
                                ev = eb[:kn].rearrange(
                                    "k (p q) -> k p q", p=2
                                )[:, :, :qn]
                                nc.vector.tensor_scalar(
                                    ev, sv, A_SCH, B_SCH,
                                    mybir.AluOpType.mult, mybir.AluOpType.add,
                                )
                                es.append(eb[:].bitcast(BF16))
                            else:
                                e = epool.tile(
                                    [128, 1024], BF16, tag="e", name="e"
                                )
                                ev = e[:kn].rearrange(
                                    "k (p q) -> k p q", p=2
                                )[:, :, :qn]
                                nc.scalar.activation(
                                    ev, sv, mybir.ActivationFunctionType.Exp,
                                    scale=SCALE,
                                )
                                es.append(e[:])
                        if pend[0] is not None:
                            emit_u(*pend[0])
                        pend[0] = (pu, qn, kc, kn, es, kc == NKC - 1)
                        work = boundary_work.get(kc)
                        if work:
                            work()
                    return pu

                def tail_a(qt, pu):
                    """ACT 1/Z (ln, exp(-x); same table set) + row-dup DMAs."""
                    qo, qn = QTS[qt]
                    lnz = tailpool.tile([128, 512], F32, tag="lnz", name="lnz")
                    zs = tailpool.tile([128, 512], F32, tag="zs", name="zs")
                    nc.scalar.activation(
                        lnz[:, :qn], pu[:, :qn],
                        mybir.ActivationFunctionType.Ln,
                    )
                    nc.scalar.activation(
                        zs[:, :qn], lnz[:, :qn],
                        mybir.ActivationFunctionType.Exp, scale=-1.0,
                    )
                    for h in range(HLOC):
                        src = (
                            zs[32 * h + 2 : 32 * h + 3, :qn]
                            .unsqueeze(1)
                            .broadcast_to([1, 2, qn])
                        )
                        dma = (
                            nc.sync.dma_start if h % 2 == 0
                            else nc.gpsimd.dma_start
                        )
                        dma(
                            out=zz[32 * h : 32 * h + 2, qo : qo + qn], in_=src
                        )

                def tail_b(qt, pu):
                    """Normalize (DVE mul), proj matmul, bias add, flush."""
                    qo, qn = QTS[qt]
                    nc.vector.tensor_tensor(
                        osp[:, qo : qo + qn], pu[:, :qn], zz[:, qo : qo + qn],
                        mybir.AluOpType.mult,
                    )
                    ytp = ps_m.tile([128, 512], F32, tag="pu", name="ytp")
                    nc.tensor.matmul(
                        ytp[0:C, :qn], wp_sb[:], osp[:, qo : qo + qn],
                        start=True, stop=True,
                    )
                    nc.vector.tensor_tensor(
                        yt_sb[:, qo : qo + qn], ytp[0:C, :qn],
                        b_sb[:].broadcast_to([C, qn]),
                        mybir.AluOpType.add,
                    )
                    nc.sync.dma_start(
                        out=yt[:, qo : qo + qn], in_=yt_sb[:, qo : qo + qn]
                    )

                def qk_more(which, dst8, dst, seg):
                    qkv_seg(which, dst8, seg)
                    band_dist(dst, dst8, *SEGS[seg])

                tails = [None] * 4
                bw0 = {
                    0: lambda: qk_more("k", k8, kT, 1),
                    1: lambda: qk_more("k", k8, kT, 2),
                    2: lambda: qk_more("k", k8, kT, 3),
                    5: lambda: qk_more("q", q8, qT, 1),
                    8: lambda: qk_more("q", q8, qT, 2),
                    11: lambda: qk_more("q", q8, qT, 3),
                }
                tails[0] = qtile(0, bw0)
                bw = {
                    0: lambda: tail_a(0, tails[0]),
                    3: lambda: tail_b(0, tails[0]),
                }
                tails[1] = qtile(1, bw)
                bw = {
                    0: lambda: tail_a(1, tails[1]),
                    3: lambda: tail_b(1, tails[1]),
                }
                tails[2] = qtile(2, bw)
                bw = {
                    0: lambda: tail_a(2, tails[2]),
                    3: lambda: tail_b(2, tails[2]),
                }
                tails[3] = qtile(3, bw)
                emit_u(*pend[0])
                pend[0] = None
                tail_a(3, tails[3])
                tail_b(3, tails[3])

            init()
            if reps <= 4:
                for _ in range(reps):
                    body()
            else:
                with tc.For_i(
                    0, reps, 1, hint_engines=(mybir.EngineType.PE,)
                ):
                    body()

    return nc


_NC = None


def _finalize(nc):
    """Finalize with the activation-table list narrowed to
    natural_log_exp_and_others (contains Exp, Ln, Copy — everything this
    kernel runs on ScalarE), so the table-load pass emits ONE load instead
    of thrashing between the exp and ln sets at every query-tile tail."""
    import concourse.bacc as _bacc

    orig = _bacc.get_activation_tables

    def only_ln_exp(arch):
        tabs = orig(arch)
        if "natural_log_exp_and_others" not in tabs:
            return tabs
        out = {}
        for k, v in tabs.items():
            if k != "natural_log_exp_and_others":
                v = {
                    f for f in v
                    if f not in (
                        mybir.ActivationFunctionType.Exp,
                        mybir.ActivationFunctionType.Ln,
                    )
                }
            out[k] = v
        return out

    _bacc.get_activation_tables = only_ln_exp
    try:
        nc.finalize()
    finally:
        _bacc.get_activation_tables = orig
    return nc


def _get_nc():
    global _NC
    if _NC is None:
        _NC = _finalize(build_nc())
    return _NC


def make_in_maps(x, w_qkv, w_proj, b_proj):
    x2 = np.ascontiguousarray(x.reshape(C, N)).astype(ml_dtypes.bfloat16)
    in_maps = []
    for c in range(NCORES):
        sl = slice(8 * c, 8 * c + 8)
        wq = w_qkv[sl, :].T                # [64, 8]
        wk = w_qkv[64 + 8 * c : 64 + 8 * c + 8, :].T
        wv = w_qkv[128 + 8 * c : 128 + 8 * c + 8, :].T
        wqkv_c = np.concatenate([wq, wk, wv], axis=1).astype(ml_dtypes.bfloat16)
        wpa = np.zeros((128, C), np.float32)
        for h in range(HLOC):
            for d in range(2):
                wpa[32 * h + d, :] = w_proj[:, 8 * c + 2 * h + d]
        in_maps.append(
            {
                "x2": x2,
                "wqkv": np.ascontiguousarray(wqkv_c),
                "wpa": np.ascontiguousarray(wpa.astype(ml_dtypes.bfloat16)),
                "bcol": np.ascontiguousarray(
                    (b_proj / NCORES)[:, None].astype(np.float32)
                ),
            }
        )
    return in_maps


def run(x, w_qkv, w_proj, b_proj, trace=False, **kw):
    nc = _get_nc()
    in_maps = make_in_maps(x, w_qkv, w_proj, b_proj)
    res = run_bass_kernel_spmd(
        nc, in_maps, core_ids=list(range(NCORES)), trace=trace, **kw
    )
    yt_sum = np.zeros((C, N), np.float32)
    for r in res.results:
        yt_sum += r["yt"]
    return yt_sum.T.reshape(1, 12, 12, 12, C), res


def kernel(x, w_qkv, w_proj, b_proj):
    out, _ = run(
        np.asarray(x), np.asarray(w_qkv), np.asarray(w_proj), np.asarray(b_proj)
    )
    return out


# revision 29
# speedup vs baseline: 1.0692x; 1.0692x over previous
"""Trainium2 Bass kernel v2 for nn_Attention (B=1, C=64, 12^3 spatial, 32 heads, d=2).

Sharding: 32 heads over 8 cores (4 heads/core), tensor-parallel w_proj with
host-side partial sum (no collectives).

Key structure per core (vs v1 baseline):
- exp work split between ScalarE (true exp activation) and VectorE
  (Schraudolph exp2 bit-trick: int16(s*A+B) bitcast to bf16, ~3% rel err on
  a tunable fraction of key chunks) so the two engines share the 12M-element
  softmax exponential instead of serializing on ScalarE.
- S tiles pack a head PAIR per PSUM tile [128, 2x512] so one exp instruction
  covers 1024 elements/partition (fewer, larger activations).
- U/Z accumulated flash-style via V'=[V0,V1,1] matmuls; PSUM pre-zeroed by a
  zero-weight matmul (PE) so downstream full-tile reads are initialized.
- 1/Z via ACT ln then exp(-x) (same activation table set as exp -> no
  table switch), duplicated to the two U rows per head by broadcast DMAs.
- proj computed as y^T = Wp'^T @ O_sp (stationary Wp', moving normalized O),
  bias added via broadcast tensor_tensor; output is y^T [64, N] summed on host.
- a dummy exp right after startup forces the ACT table load off the
  critical path.

reps>1 wraps the body in a hardware For_i loop (HW timing measurements).

Self-contained: hardcodes all shapes.
"""

import numpy as np
import ml_dtypes

import concourse.bass as bass
import concourse.bacc as bacc
import concourse.mybir as mybir
from concourse import tile
from concourse.bass_utils import run_bass_kernel_spmd

C = 64
N = 1728  # 12*12*12
NCORES = 8
HLOC = 4
SCALE = float(2.0 ** -0.5)
LOG2E = float(np.log2(np.e))
# Schraudolph bf16-exp2 constants: bits_i16 = s*A + B, bits viewed as bf16
A_SCH = SCALE * LOG2E * 128.0
B_SCH = (127.0 - 0.04367) * 128.0 + 0.5

KCS = [(i * 128, 128) for i in range(13)] + [(1664, 64)]
NKC = len(KCS)
QTS = [(0, 512), (512, 512), (1024, 512), (1536, 192)]
SEGS = [(0, 512), (512, 512), (1024, 512), (1536, 192)]  # qkv segments

F32 = mybir.dt.float32
BF16 = mybir.dt.bfloat16
I16 = mybir.dt.int16

F_DVE = 0.45      # fraction of exp tiles on VectorE (Schraudolph)
QK_ACT = 0.5      # fraction of qk PSUM->SBUF copies on ScalarE


def _mk_assign(n, frac):
    """Error-diffused boolean assignment list: True -> alt engine."""
    out, acc = [], 0.0
    for _ in range(n):
        acc += frac
        if acc >= 1.0 - 1e-9:
            acc -= 1.0
            out.append(True)
        else:
            out.append(False)
    return out


def build_nc(f_dve=F_DVE, qk_act=QK_ACT, reps=1):
    nc = bacc.Bacc(None)

    x2 = nc.declare_dram_parameter("x2", [C, N], BF16, isOutput=False)
    wqkv = nc.declare_dram_parameter("wqkv", [C, 24], BF16, isOutput=False)
    wpa = nc.declare_dram_parameter("wpa", [128, C], BF16, isOutput=False)
    bcol = nc.declare_dram_parameter("bcol", [C, 1], F32, isOutput=False)
    yt = nc.declare_dram_parameter("yt", [C, N], F32, isOutput=True)

    with tile.TileContext(nc) as tc:
        with (
            tc.tile_pool(name="const", bufs=1) as cpool,
            tc.tile_pool(name="epool", bufs=5) as epool,
            tc.tile_pool(name="ebpool", bufs=5) as ebpool,
            tc.tile_pool(name="tails", bufs=2) as tailpool,
            tc.tile_pool(name="ps_s", bufs=3, space=bass.MemorySpace.PSUM) as ps_s,
            tc.tile_pool(name="ps_u", bufs=2, space=bass.MemorySpace.PSUM) as ps_u,
        ):
            ps_m = ps_u  # pu/psv/qkv/ytp share the two 1-bank slots
            x_sb = cpool.tile([C, N], BF16, name="x_sb")
            w_sb = cpool.tile([C, 24], BF16, name="w_sb")
            wp_sb = cpool.tile([128, C], BF16, name="wp_sb")
            b_sb = cpool.tile([C, 1], F32, name="b_sb")
            qT = cpool.tile([128, N], BF16, name="qT")
            kT = cpool.tile([128, N], BF16, name="kT")
            q8 = cpool.tile([8, N], BF16, name="q8")
            k8 = cpool.tile([8, N], BF16, name="k8")
            vp = cpool.tile([128, NKC * HLOC * 3], BF16, name="vp")
            zrow = cpool.tile([1, 128], BF16, name="zrow")
            zdum = cpool.tile([1, 512], BF16, name="zdum")
            scr = cpool.tile([1, 8], F32, name="scr")
            zz = cpool.tile([128, N], F32, name="zz")
            osp = cpool.tile([128, N], BF16, name="osp")
            yt_sb = cpool.tile([C, N], F32, name="yt_sb")
            vp_v = vp[:].rearrange("p (kc h d) -> p kc h d", h=HLOC, d=3)

            def init():
                # one-time constants, hoisted out of the rep loop
                nc.gpsimd.memset(zrow[:], 0.0)
                nc.gpsimd.memset(zdum[:], 0.0)
                # dummy exp: forces the ACT table load at t~0
                nc.scalar.activation(
                    scr[0:1, 0:2], zrow[0:1, 0:2],
                    mybir.ActivationFunctionType.Exp,
                )
                nc.sync.dma_start(out=w_sb[:], in_=wqkv[:])
                nc.sync.dma_start(out=wp_sb[:], in_=wpa[:])
                nc.sync.dma_start(out=b_sb[:], in_=bcol[:])
                # zz rows outside the dup-DMA bands stay zero forever; the
                # vp ones-column is never touched by the per-iter copies
                nc.gpsimd.memset(zz[:], 0.0)
                nc.gpsimd.memset(vp_v[:, :, :, 2:3], 1.0)

            def body():
                exp_assign = _mk_assign(len(QTS) * NKC * 2, f_dve)
                qk_assign = _mk_assign(2 * len(SEGS), qk_act)
                exp_i = [0]
                qk_i = [0]

                nc.sync.dma_start(out=x_sb[:, 0:512], in_=x2[:, 0:512])
                nc.sync.dma_start(out=x_sb[:, 512:N], in_=x2[:, 512:N])

                # ---- V': per key chunk, v rows [kn, 8] -> vp packed bf16
                # (deferred into qtile0's chunk stream: vp is first needed by
                # emit_u during chunk 1, so the 14 LDW-heavy matmuls should
                # not serialize ahead of the first scores matmuls) ----
                def build_vp():
                    psv = ps_m.tile([128, 512], F32, tag="pu", name="psv")
                    for kc, (ko, kn) in enumerate(KCS):
                        nc.tensor.matmul(
                            psv[:kn, 8 * kc : 8 * kc + 8],
                            x_sb[:, ko : ko + kn],
                            w_sb[:, 16:24],
                            start=True, stop=True,
                        )
                    vsrc = psv[:, 0:104].rearrange(
                        "p (kc h d) -> p kc h d", h=HLOC, d=2
                    )
                    nc.vector.tensor_copy(vp_v[:, 0:13, :, 0:2], vsrc)
                    vtail = psv[:64, 104:112].rearrange("p (h d) -> p h d", d=2)
                    nc.vector.tensor_copy(vp_v[:64, 13, :, 0:2], vtail)

                # ---- qkv: one [8, segn] matmul per seg; stage to an [8, N]
                # sbuf tile, then DMA-distribute head rows 2h -> band 32h ----
                def qkv_seg(which, dst8, seg):
                    so, sn = SEGS[seg]
                    wofs = 0 if which == "q" else 8
                    ps = ps_m.tile([128, 512], F32, tag="pu", name="ps_qk")
                    nc.tensor.matmul(
                        ps[0:8, :sn],
                        w_sb[:, wofs : wofs + 8],
                        x_sb[:, so : so + sn],
                        start=True, stop=True,
                    )
                    on_act = qk_assign[qk_i[0]]
                    qk_i[0] += 1
                    src = ps[0:8, :sn]
                    dd = dst8[0:8, so : so + sn]
                    if on_act:
                        nc.scalar.copy(dd, src)
                    else:
                        nc.vector.tensor_copy(dd, src)

                def band_dist(dst, src8, so, sn):
                    """DMA rows 2h:2h+2 of src8 -> rows 32h:32h+2 of dst."""
                    for h in range(HLOC):
                        dma = nc.sync.dma_start if h % 2 == 0 else nc.gpsimd.dma_start
                        dma(
                            out=dst[32 * h : 32 * h + 2, so : so + sn],
                            in_=src8[2 * h : 2 * h + 2, so : so + sn],
                        )

                qkv_seg("k", k8, 0)
                qkv_seg("q", q8, 0)
                band_dist(kT, k8, 0, 512)
                band_dist(qT, q8, 0, 512)

                # U matmuls run one key-chunk behind the exps — and the
                # pipeline carries ACROSS qtile boundaries — so the in-order
                # PE queue never fences on the current chunk's exp.
                pend = [None]  # (pu, qn, kc, kn, es, last)

                def emit_u(pu, qn, kc, kn, es, last):
                    for h in range(HLOC):
                        nc.tensor.matmul(
                            pu[32 * h : 32 * h + 3, :qn],
                            vp_v[:kn, kc, h, :],
                            es[h // 2][
                                :kn, 512 * (h % 2) : 512 * (h % 2) + qn
                            ],
                            start=False,
                            stop=(last and h == HLOC - 1),
                            skip_group_check=True,
                            tile_position=(0, 32 * h),
                        )

                def qtile(qt, boundary_work):
                    qo, qn = QTS[qt]
                    pu = ps_u.tile([128, 512], F32, tag="pu", name="pu")
                    # zero the U bank via a zero-weight matmul (starts group)
                    nc.tensor.matmul(
                        pu[:, :qn], zrow[:], zdum[0:1, :qn],
                        start=True, stop=False, skip_group_check=True,
                    )
                    for kc, (ko, kn) in enumerate(KCS):
                        es = []
                        for pr in range(2):
                            ha, hb = 2 * pr, 2 * pr + 1
                            st = ps_s.tile([128, 1024], F32, tag="s", name="st")
                            nc.tensor.matmul(
                                st[:kn, 0:qn],
                                kT[32 * ha : 32 * ha + 2, ko : ko + kn],
                                qT[32 * ha : 32 * ha + 2, qo : qo + qn],
                                start=True, stop=True,
                                tile_position=(32 * ha, 0),
                            )
                            nc.tensor.matmul(
                                st[:kn, 512 : 512 + qn],
                                kT[32 * hb : 32 * hb + 2, ko : ko + kn],
                                qT[32 * hb : 32 * hb + 2, qo : qo + qn],
                                start=True, stop=True,
                                tile_position=(32 * hb, 0),
                            )
                            on_dve = exp_assign[exp_i[0]]
                            exp_i[0] += 1
                            sv = st[:kn].rearrange("k (p q) -> k p q", p=2)[
                                :, :, :qn
                            ]
                            if on_dve:
                                eb = ebpool.tile(
                                    [128, 1024], I16, tag="eb", name="eb"
                                )
                                ev = eb[:kn].rearrange(
                                    "k (p q) -> k p q", p=2
                                )[:, :, :qn]
                                nc.vector.tensor_scalar(
                                    ev, sv, A_SCH, B_SCH,
                                    mybir.AluOpType.mult, mybir.AluOpType.add,
                                )
                                es.append(eb[:].bitcast(BF16))
                            else:
                                e = epool.tile(
                                    [128, 1024], BF16, tag="e", name="e"
                                )
                                ev = e[:kn].rearrange(
                                    "k (p q) -> k p q", p=2
                                )[:, :, :qn]
                                nc.scalar.activation(
                                    ev, sv, mybir.ActivationFunctionType.Exp,
                                    scale=SCALE,
                                )
                                es.append(e[:])
                        if pend[0] is not None:
                            emit_u(*pend[0])
                        pend[0] = (pu, qn, kc, kn, es, kc == NKC - 1)
                        work = boundary_work.get(kc)
                        if work:
                            work()
                    return pu

                def tail_a(qt, pu):
                    """ACT 1/Z (ln, exp(-x); same table set) + row-dup DMAs."""
                    qo, qn = QTS[qt]
                    lnz = tailpool.tile([128, 512], F32, tag="lnz", name="lnz")
                    zs = tailpool.tile([128, 512], F32, tag="zs", name="zs")
                    nc.scalar.activation(
                        lnz[:, :qn], pu[:, :qn],
                        mybir.ActivationFunctionType.Ln,
                    )
                    nc.scalar.activation(
                        zs[:, :qn], lnz[:, :qn],
                        mybir.ActivationFunctionType.Exp, scale=-1.0,
                    )
                    for h in range(HLOC):
                        src = (
                            zs[32 * h + 2 : 32 * h + 3, :qn]
                            .unsqueeze(1)
                            .broadcast_to([1, 2, qn])
                        )
                        dma = (
                            nc.sync.dma_start if h % 2 == 0
                            else nc.gpsimd.dma_start
                        )
                        dma(
                            out=zz[32 * h : 32 * h + 2, qo : qo + qn], in_=src
                        )

                def tail_b(qt, pu):
                    """Normalize (DVE mul), proj matmul, bias add, flush."""
                    qo, qn = QTS[qt]
                    nc.vector.tensor_tensor(
                        osp[:, qo : qo + qn], pu[:, :qn], zz[:, qo : qo + qn],
                        mybir.AluOpType.mult,
                    )
                    ytp = ps_m.tile([128, 512], F32, tag="pu", name="ytp")
                    nc.tensor.matmul(
                        ytp[0:C, :qn], wp_sb[:], osp[:, qo : qo + qn],
                        start=True, stop=True,
                    )
                    nc.vector.tensor_tensor(
                        yt_sb[:, qo : qo + qn], ytp[0:C, :qn],
                        b_sb[:].broadcast_to([C, qn]),
                        mybir.AluOpType.add,
                    )
                    nc.sync.dma_start(
                        out=yt[:, qo : qo + qn], in_=yt_sb[:, qo : qo + qn]
                    )

                def qk_more(which, dst8, dst, seg):
                    qkv_seg(which, dst8, seg)
                    band_dist(dst, dst8, *SEGS[seg])

                tails = [None] * 4
                bw0 = {
                    0: lambda: (build_vp(), qk_more("k", k8, kT, 1)),
                    1: lambda: qk_more("k", k8, kT, 2),
                    2: lambda: qk_more("k", k8, kT, 3),
                    5: lambda: qk_more("q", q8, qT, 1),
                    8: lambda: qk_more("q", q8, qT, 2),
                    11: lambda: qk_more("q", q8, qT, 3),
                }
                tails[0] = qtile(0, bw0)
                bw = {
                    0: lambda: tail_a(0, tails[0]),
                    3: lambda: tail_b(0, tails[0]),
                }
                tails[1] = qtile(1, bw)
                bw = {
                    0: lambda: tail_a(1, tails[1]),
                    3: lambda: tail_b(1, tails[1]),
                }
                tails[2] = qtile(2, bw)
                bw = {
                    0: lambda: tail_a(2, tails[2]),
                    3: lambda: tail_b(2, tails[2]),
                }
                tails[3] = qtile(3, bw)
                emit_u(*pend[0])
                pend[0] = None
                tail_a(3, tails[3])
                tail_b(3, tails[3])

            init()
            if reps <= 4:
                for _ in range(reps):
                    body()
            else:
                with tc.For_i(
                    0, reps, 1, hint_engines=(mybir.EngineType.PE,)
                ):
                    body()

    return nc


_NC = None


def _finalize(nc):
    """Finalize with the activation-table list narrowed to
    natural_log_exp_and_others (contains Exp, Ln, Copy — everything this
    kernel runs on ScalarE), so the table-load pass emits ONE load instead
    of thrashing between the exp and ln sets at every query-tile tail."""
    import concourse.bacc as _bacc

    orig = _bacc.get_activation_tables

    def only_ln_exp(arch):
        tabs = orig(arch)
        if "natural_log_exp_and_others" not in tabs:
            return tabs
        out = {}
        for k, v in tabs.items():
            if k != "natural_log_exp_and_others":
                v = {
                    f for f in v
                    if f not in (
                        mybir.ActivationFunctionType.Exp,
                        mybir.ActivationFunctionType.Ln,
                    )
                }
            out[k] = v
        return out

    _bacc.get_activation_tables = only_ln_exp
    try:
        nc.finalize()
    finally:
        _bacc.get_activation_tables = orig
    return nc


def _get_nc():
    global _NC
    if _NC is None:
        _NC = _finalize(build_nc())
    return _NC


def make_in_maps(x, w_qkv, w_proj, b_proj):
    x2 = np.ascontiguousarray(x.reshape(C, N)).astype(ml_dtypes.bfloat16)
    in_maps = []
    for c in range(NCORES):
        sl = slice(8 * c, 8 * c + 8)
        wq = w_qkv[sl, :].T                # [64, 8]
        wk = w_qkv[64 + 8 * c : 64 + 8 * c + 8, :].T
        wv = w_qkv[128 + 8 * c : 128 + 8 * c + 8, :].T
        wqkv_c = np.concatenate([wq, wk, wv], axis=1).astype(ml_dtypes.bfloat16)
        wpa = np.zeros((128, C), np.float32)
        for h in range(HLOC):
            for d in range(2):
                wpa[32 * h + d, :] = w_proj[:, 8 * c + 2 * h + d]
        in_maps.append(
            {
                "x2": x2,
                "wqkv": np.ascontiguousarray(wqkv_c),
                "wpa": np.ascontiguousarray(wpa.astype(ml_dtypes.bfloat16)),
                "bcol": np.ascontiguousarray(
                    (b_proj / NCORES)[:, None].astype(np.float32)
                ),
            }
        )
    return in_maps


def run(x, w_qkv, w_proj, b_proj, trace=False, **kw):
    nc = _get_nc()
    in_maps = make_in_maps(x, w_qkv, w_proj, b_proj)
    res = run_bass_kernel_spmd(
        nc, in_maps, core_ids=list(range(NCORES)), trace=trace, **kw
    )
    yt_sum = np.zeros((C, N), np.float32)
    for r in res.results:
        yt_sum += r["yt"]
    return yt_sum.T.reshape(1, 12, 12, 12, C), res


def kernel(x, w_qkv, w_proj, b_proj):
    out, _ = run(
        np.asarray(x), np.asarray(w_qkv), np.asarray(w_proj), np.asarray(b_proj)
    )
    return out
